# revision 13
# baseline (speedup 1.0000x reference)
"""NetVLAD on 8 Trainium2 NeuronCores — self-contained kernel.

Problem: x [32, 2048, 1024] f32, W [64, 1024] f32, centroids [64, 1024] f32
  -> out [32, 65536] f32  (NetVLAD pooling: per-frame L2 norm, soft-assign
  softmax over 64 clusters, residual aggregation, intra + global L2 norm).

Sharding: data-parallel over batch — 4 samples per core, W/centroids
replicated; no cross-core communication.

Final design (~204us vs 314us baseline):
  - x ingested via SWDGE casting DMA f32->fp8e4 per quarter (read-bound at
    the HBM/SDMA ceiling; zero engine-side cast work).
  - ONE xbar DMA-transpose per quarter of the uint16(fp8-pair)-viewed data
    (half the bytes of bf16), issue split across both HWDGE queues
    (sync/scalar).
  - logits via fp8 DoubleRow matmuls (4 per m-tile, Ki=128 x Ko=2 U-blocks,
    byte offset selects the fp8 half of each transposed uint16); W goes
    through the identical cast+transpose transform so contraction pairs
    line up.
  - a (fp8) scaled x256 and invr (fp8) scaled /4 to stay in e4m3's normal
    range; the factors cancel in the tail normalizations (cs rescaled x4).
  - ssq split ACT Square+accum (2 tiles) / DVE tensor_mul+reduce (2 tiles);
    ACT restricted to {Ln, Exp, Square, Copy} (one table set).
  - BIR post-pass _overlap_dma_json: un-pins the scheduler-serialized DMA
    queues (loads/transposes overlap on the 16 SDMA engines) and paces
    loads 4 quarters ahead of transposes.
"""

import json

import numpy as np

import concourse.bass as bass
import concourse.mybir as mybir
import concourse.tile as tile

F32 = mybir.dt.float32
BF16 = mybir.dt.bfloat16
FP8 = mybir.dt.float8e4
AF = mybir.ActivationFunctionType
OP = mybir.AluOpType

B = 32
N_CORES = 8
B_PER_CORE = B // N_CORES
M = 2048
D = 1024
K = 64
NQ = 4           # quarters per sample
TQ = 4           # m-tiles per quarter

_PATCHED = False


def _prune_teardown_waits_json(bir: dict) -> dict:
    """Drop DMASW-lane waits from teardown-block Drain instructions.

    The kernel-end Drain on SP waits on every DMA lane; after wait-splitting
    each wait becomes a serial ~115ns EventSemaphore. The DMASW lanes track
    the gpsimd casting loads, every one of which was already consumed by an
    xbar transpose (whose issue waited on that lane value), so by teardown
    time those lanes are provably at their final values — the waits are pure
    dispatch overhead on the critical tail.
    """
    for f in bir.get("functions", []):
        blocks = f.get("blocks", [])
        for blk in blocks[1:]:
            for inst in blk.get("instructions", []):
                if inst.get("opcode") != "Drain":
                    continue
                si = inst.get("sync_info")
                if not si:
                    continue
                si["on_wait"] = [
                    w for w in si.get("on_wait", [])
                    if not w["ant_name"].startswith("DMASW")
                ]
    return bir


def _split_waits_json(bir: dict, max_waits: int = 1) -> dict:
    """Split multi-wait sync infos into standalone EventSemaphore waits.

    The walrus build in this image supports a single sync-wait command per
    instruction, while Tile's sem assignment emits several (e.g. the
    kernel-tail Drain waits on every DMAHW lane). Hoisting the extra waits
    into preceding single-wait EventSemaphore instructions on the same
    engine is semantics-preserving for monotonic semaphores.
    """
    ctr = 0
    for f in bir.get("functions", []):
        for blk in f.get("blocks", []):
            insts = blk.get("instructions", [])
            new = []
            for inst in insts:
                si = inst.get("sync_info")
                waits = si.get("on_wait", []) if si else []
                if len(waits) > max_waits:
                    head, keep = waits[:-max_waits], waits[-max_waits:]
                    for w in head:
                        ctr += 1
                        new.append({
                            "debug": inst.get("debug", 0),
                            "engine": inst["engine"],
                            "ins": [],
                            "name": f"{inst['name']}-wsplit{ctr}",
                            "opcode": "EventSemaphore",
                            "outs": [],
                            "sync_info": {"on_update": [], "on_wait": [w]},
                        })
                    si["on_wait"] = keep
                new.append(inst)
            blk["instructions"] = new
    return bir


def _overlap_dma_json(bir: dict) -> dict:
    """Un-serialize the x-load and xbar-transpose DMA queues.

    The Tile scheduler's cost model holds a single modeled DMA_ENGINES
    resource exclusively for every DMA, so its schedule lays all DMAs out in
    one serial chain and the semaphore pass pins that order with cross-queue
    waits (x-loads wait on recent transposes and vice versa). On hardware the
    16 SDMA engines serve both queues concurrently, so those pins only create
    idle time (measured as strict alternation of the two queues, ~30% SDMA
    idle).

    Safe rewrite, relying on xb pool bufs == number of quarters (each xb
    buffer is written exactly once, so an x-load has NO data hazard at all):
      - drop every DMAHW-lane wait from x-load DMACopy instructions (they can
        only be scheduler order-pins);
      - on each x-transpose, replace the DMASW-lane waits with the one true
        dependency: the completion value of its own quarter's x-load
        (tracked by walking lane update counts in program order). PE waits
        (xT buffer reuse) and DMAHW waits (transpose queue ordering) are
        kept.
    """
    PACE_DEPTH = 4
    for f in bir.get("functions", []):
        for blk in f.get("blocks", []):
            lane_val: dict[str, int] = {}
            load_done: dict[str, tuple[dict, int]] = {}
            tp_updates: list[tuple[dict, int]] = []
            n_loads = 0
            for inst in blk.get("instructions", []):
                si = inst.get("sync_info")
                if si is None:
                    continue
                outs = inst.get("outs") or []
                ins = inst.get("ins") or []
                outm = outs[0].get("memref", "") if outs else ""
                inm = ins[0].get("memref", "") if ins else ""
                is_xload = (
                    inst.get("opcode") == "DMACopy"
                    and inst.get("engine") == "Pool"
                    and outm.startswith("xb_")
                )
                is_xtp = (
                    inst.get("opcode") == "DmaTransposeAnt"
                    and inm.startswith("xb_")
                )
                if is_xload:
                    waits = [
                        w for w in si.get("on_wait", [])
                        if not w["ant_name"].startswith("DMAHW")
                    ]
                    # Pace the load stream: the 4KB-read load packets starve
                    # the 256B xbar packets in the SDMA round-robin, so gate
                    # load n on transpose n-PACE_DEPTH to keep the two
                    # streams in lockstep (idle SDMA slots go to transposes).
                    idx = n_loads - PACE_DEPTH
                    if 0 <= idx < len(tp_updates):
                        upd, val = tp_updates[idx]
                        waits.append({
                            "ant_name": upd["ant_name"],
                            "id": upd["id"],
                            "sync_type": "semaphore",
                            "wait_mode": "sem-ge-imm",
                            "wait_value": val,
                        })
                    si["on_wait"] = waits
                    n_loads += 1
                if is_xtp:
                    key = "_".join(inm.split("_")[:3])
                    keep = [
                        w for w in si.get("on_wait", [])
                        if not w["ant_name"].startswith("DMASW")
                    ]
                    real = load_done.get(key)
                    if real is not None:
                        upd, val = real
                        keep.append({
                            "ant_name": upd["ant_name"],
                            "id": upd["id"],
                            "sync_type": "semaphore",
                            "wait_mode": "sem-ge-imm",
                            "wait_value": val,
                        })
                    si["on_wait"] = keep
                for u in si.get("on_update", []):
                    nm = u.get("ant_name")
                    if not nm:
                        continue
                    lane_val[nm] = lane_val.get(nm, 0) + u.get(
                        "update_value", 0)
                    if is_xload:
                        key = "_".join(outm.split("_")[:3])
                        load_done[key] = (u, lane_val[nm])
                    if is_xtp:
                        tp_updates.append((u, lane_val[nm]))
    return bir


def _apply_patch():
    global _PATCHED
    if _PATCHED:
        return
    import concourse.bass_utils as bu
    import concourse.bass2jax as b2j
    orig = bu.compile_bir_kernel

    def patched(bir_json, tmpdir, neff_name="file.neff"):
        d = json.loads(bir_json)
        d = _overlap_dma_json(d)
        d = _prune_teardown_waits_json(d)
        d = _split_waits_json(d, 1)
        return orig(json.dumps(d).encode(), tmpdir, neff_name)

    bu.compile_bir_kernel = patched
    b2j.compile_bir_kernel = patched
    _PATCHED = True


def build_nc():
    nc = bass.Bass()
    x = nc.dram_tensor("x", [B_PER_CORE, M, D], F32, kind="ExternalInput")
    W = nc.dram_tensor("W", [K, D], F32, kind="ExternalInput")
    C = nc.dram_tensor("centroids", [K, D], F32, kind="ExternalInput")
    out = nc.dram_tensor("out", [B_PER_CORE, K * D], F32, kind="ExternalOutput")
    ind2_d = nc.dram_tensor("ind2", [2, 128], F32, kind="ExternalInput")
    indK_d = nc.dram_tensor("indK", [128, 2], F32, kind="ExternalInput")

    # partition p holds 4 contiguous rows (m = q*512 + 4p + j): one 16KB
    # contiguous read per partition per quarter-load instead of 4x4KB.
    # NetVLAD aggregates over m, so this row relabeling is output-invariant.
    xr = x[:, :, :].rearrange("s (q t p) d -> s q p t d", q=NQ, t=TQ, p=128)
    outr = out[:, :].rearrange("s (k d) -> s k d", d=D)

    from contextlib import ExitStack
    with tile.TileContext(nc) as tc, ExitStack() as es:
        singles = es.enter_context(tc.tile_pool(name="singles", bufs=1))
        # xb bufs=16: every quarter gets its own buffer (written exactly once)
        # so x-loads carry no write-after-read hazard at all — this makes the
        # BIR-level wait surgery in _overlap_dma_json provably safe.
        xbpool = es.enter_context(tc.tile_pool(name="xbp", bufs=16))
        xTpool = es.enter_context(tc.tile_pool(name="xTp", bufs=8))
        sqpool = es.enter_context(tc.tile_pool(name="sqp", bufs=3))
        statpool = es.enter_context(tc.tile_pool(name="statp", bufs=8))
        epool = es.enter_context(tc.tile_pool(name="ep", bufs=4))
        apool = es.enter_context(tc.tile_pool(name="apl", bufs=4))
        rspool = es.enter_context(tc.tile_pool(name="rsp", bufs=4))
        tailpool = es.enter_context(tc.tile_pool(name="tailp", bufs=1))
        zpsum = es.enter_context(tc.tile_pool(name="zps", bufs=2, space="PSUM"))
        aggpsum = es.enter_context(
            tc.tile_pool(name="aggps", bufs=2, space="PSUM"))
        cspsum = es.enter_context(tc.tile_pool(name="csps", bufs=1, space="PSUM"))
        tailpsum = es.enter_context(tc.tile_pool(name="tps", bufs=1, space="PSUM"))

        # W: fp8-cast then the same uint16-viewed xbar transpose as x, so the
        # fp8 pair layout (d = 256V + 2p + b at [p, V, :, b]) matches x's and
        # DoubleRow contraction pairs line up on both operands.
        w8 = singles.tile([K, D], FP8)
        nc.gpsimd.dma_start(out=w8, in_=W[:, :])
        WT16 = singles.tile([128, 4, K], BF16)  # [p, V, k] uint16 pairs
        nc.sync.dma_start(out=WT16, in_=w8[:, :].bitcast(BF16), transpose=True)
        cpair = singles.tile([128, D], F32)
        nc.gpsimd.dma_start(out=cpair[0:64, :], in_=C[:, :])
        nc.gpsimd.dma_start(out=cpair[64:128, :], in_=C[:, :])
        ind2 = singles.tile([2, 128], F32)
        nc.sync.dma_start(out=ind2, in_=ind2_d[:, :])
        indK = singles.tile([128, 2], F32)
        nc.sync.dma_start(out=indK, in_=indK_d[:, :])


        def quarter(s, q, agg, cs):
            base = 64 * (s % 2)
            first = q == 0
            last = q == NQ - 1

            # 1) casting quarter load (SWDGE): f32 DRAM -> fp8e4 SBUF
            #    (tile names keep the xb_ prefix: _overlap_dma_json keys on it)
            xb = xbpool.tile([128, TQ, D], FP8, tag="xb", name=f"xb_{s}_{q}")
            nc.gpsimd.dma_start(out=xb, in_=xr[s, q])
            xb_flat = xb[:, :, :].rearrange("p t d -> p (t d)")

            # 2) per-quarter xbar transpose of the uint16(=fp8 pair) view.
            #    xT16[p, tU, m] = pair (x[m, d], x[m, d+1]) with
            #    d = t*1024 + 256*U + 2p, where tU = 4t + U.
            xT16 = xTpool.tile([128, TQ, 4, 128], BF16, tag="xT",
                               name=f"xT_{s}_{q}")
            xT3 = xT16[:, :, :, :].rearrange("p t u m -> p (t u) m")
            # all transposes on the sync queue: a blocking DMA_TRANSPOSE on
            # the scalar queue stalls ACT compute (zero overlap in traces)
            nc.sync.dma_start(out=xT3, in_=xb_flat.bitcast(BF16),
                              transpose=True)

            # 3) ssq: alternate the ACT/DVE split (3/1 vs 2/2) so both
            #    engines average ~4.6us/quarter (ACT square+accum is ~2x
            #    cheaper per tile than DVE's mul+reduce pair)
            nact = 3 if (NQ * s + q) % 2 == 0 else 2
            ssq = statpool.tile([128, TQ], F32, tag="ssq", name=f"ssq_{s}_{q}")
            for i in range(nact):
                sqa = sqpool.tile([128, D], BF16, tag="sqa",
                                  name=f"sqa_{s}_{q}_{i}")
                nc.scalar.activation(
                    out=sqa, in_=xb[:, i, :], func=AF.Square,
                    accum_out=ssq[:, i:i + 1],
                )
            ndve = TQ - nact
            sqd = sqpool.tile([128, ndve, D], BF16, tag="sqd",
                              name=f"sqd_{s}_{q}")
            sqd_flat = sqd[:, :, :].rearrange("p t d -> p (t d)")
            xbd = xb[:, nact:TQ, :].rearrange("p t d -> p (t d)")
            nc.vector.tensor_mul(sqd_flat, xbd, xbd)
            nc.vector.reduce_sum(
                out=ssq[:, nact:TQ], in_=sqd, axis=mybir.AxisListType.X)

            # 4) stats: lnt/r on ACT (Ln/Exp table set); invr8 = ||x||/4 =
            #    ssq*r/4 on DVE (frees two ACT exps vs the lnt-exp route)
            lnt = statpool.tile([128, TQ], F32, tag="lnt", name=f"lnt_{s}_{q}")
            nc.scalar.activation(out=lnt, in_=ssq, func=AF.Ln)
            r = statpool.tile([128, TQ], F32, tag="r", name=f"r_{s}_{q}")
            nc.scalar.activation(out=r, in_=lnt, func=AF.Exp, scale=-0.5)
            invr8 = statpool.tile([128, TQ], FP8, tag="invr",
                                  name=f"invr_{s}_{q}")
            nc.vector.scalar_tensor_tensor(
                out=invr8, in0=ssq, scalar=0.25, in1=r,
                op0=OP.mult, op1=OP.mult,
            )

            # 5) logits z[m, k] via fp8 DoubleRow matmuls: 4 per m-tile, each
            #    contracting 256 d (Ki=128 partitions x Ko=2 U-blocks), with
            #    the byte offset b selecting the fp8 half of each uint16.
            zq = zpsum.tile([128, TQ, K], F32, tag="zq", name=f"zq_{s}_{q}")
            for i in range(TQ):
                mms = [(U0, b) for U0 in (0, 2) for b in (0, 1)]
                for j, (U0, b) in enumerate(mms):
                    lhsT = xT16[:, i, U0:U0 + 2, :].bitcast(FP8).rearrange(
                        "p u (m b) -> p u m b", b=2)[:, :, :, b]
                    rhs = WT16[:, U0:U0 + 2, :].bitcast(FP8).rearrange(
                        "p v (k b) -> p v k b", b=2)[:, :, :, b]
                    nc.tensor.matmul(
                        zq[:, i, :], lhsT=lhsT, rhs=rhs,
                        start=(j == 0), stop=(j == 3),
                        perf_mode=mybir.MatmulPerfMode.DoubleRow,
                    )

            # 6) e = exp(z * r) per tile (ACT), then one batched sden reduce
            e = epool.tile([128, TQ, K], BF16, tag="e", name=f"e_{s}_{q}")
            for i in range(TQ):
                nc.scalar.activation(
                    out=e[:, i, :], in_=zq[:, i, :], func=AF.Exp,
                    scale=r[:, i:i + 1],
                )
            sden = statpool.tile([128, TQ], F32, tag="sden",
                                 name=f"sden_{s}_{q}")
            nc.vector.reduce_sum(
                out=sden, in_=e, axis=mybir.AxisListType.X)

            # 7) a = e * (256 r / sden): one reciprocal, one fused scale
            #    s = (rsd*256)*r, one broadcast multiply over all 4 tiles
            rsd = statpool.tile([128, TQ], F32, tag="rsd", name=f"rsd_{s}_{q}")
            nc.vector.reciprocal(out=rsd, in_=sden)
            sfac = statpool.tile([128, TQ], BF16, tag="sfac",
                                 name=f"sfac_{s}_{q}")
            nc.vector.scalar_tensor_tensor(
                out=sfac, in0=rsd, scalar=256.0, in1=r,
                op0=OP.mult, op1=OP.mult,
            )
            a = apool.tile([128, TQ, K], FP8, tag="a", name=f"a_{s}_{q}")
            nc.vector.tensor_mul(
                a[:, :, :], e[:, :, :],
                sfac[:, :].unsqueeze(-1).broadcast_to((128, TQ, K)),
            )

            # 8) agg += a^T x (x256); cs += a^T (||x||/4)  (x64)
            for i in range(TQ):
                st_ = first and i == 0
                sp_ = last and i == TQ - 1
                nc.tensor.matmul(
                    agg[base:base + 64, 0:512], lhsT=a[:, i, :],
                    rhs=xb[:, i, 0:512], start=st_, stop=sp_,
                )
                nc.tensor.matmul(
                    agg[base:base + 64, 512:1024], lhsT=a[:, i, :],
                    rhs=xb[:, i, 512:1024], start=st_, stop=sp_,
                )
                nc.tensor.matmul(
                    cs[base:base + 64, 0:1], lhsT=a[:, i, :],
                    rhs=invr8[:, i:i + 1], start=st_, stop=sp_,
                )

        def tail_pair(p, agg, cs):
            sa, sb = 2 * p, 2 * p + 1
            # agg carries x256, cs carries x64 — rescale cs by 4 here so vlad
            # = 256*(agg_true - colsum*c); the factor cancels in both norms.
            cssb = rspool.tile([128, 1], F32, tag="cssb", name=f"cssb_{p}")
            nc.vector.tensor_scalar_mul(cssb, cs[:, 0:1], 4.0)
            tmp = tailpool.tile([128, D], F32, tag="tmp", name=f"tmp_{p}")
            nc.vector.tensor_scalar_mul(tmp, cpair, cssb)
            vlad = tailpool.tile([128, D], F32, tag="vlad", name=f"vlad_{p}")
            nc.vector.tensor_sub(vlad, agg[:, :], tmp)
            sq2 = sqpool.tile([128, D], BF16, tag="sqa", name=f"sqt_{p}")
            vssq = rspool.tile([128, 1], F32, tag="vssq", name=f"vssq_{p}")
            nc.scalar.activation(out=sq2, in_=vlad, func=AF.Square,
                                 accum_out=vssq)
            lnv = rspool.tile([128, 1], F32, tag="lnv", name=f"lnv_{p}")
            nc.scalar.activation(out=lnv, in_=vssq, func=AF.Ln)
            rv = rspool.tile([128, 1], F32, tag="rv", name=f"rv_{p}")
            nc.scalar.activation(out=rv, in_=lnv, func=AF.Exp, scale=-0.5)
            ssqn = rspool.tile([128, 1], F32, tag="ssqn", name=f"ssqn_{p}")
            nc.vector.tensor_scalar(
                out=ssqn, in0=vssq, scalar1=rv, scalar2=rv,
                op0=OP.mult, op1=OP.mult,
            )
            gsum = tailpsum.tile([2, 2], F32, tag="tps", name=f"gsum_{p}")
            nc.tensor.matmul(gsum[:, 0:1], lhsT=indK, rhs=ssqn,
                             start=True, stop=True)
            lng = rspool.tile([2, 1], F32, tag="lng", name=f"lng_{p}")
            nc.scalar.activation(out=lng, in_=gsum[:, 0:1], func=AF.Ln)
            ginv = rspool.tile([2, 1], F32, tag="ginv", name=f"ginv_{p}")
            nc.scalar.activation(out=ginv, in_=lng, func=AF.Exp, scale=-0.5)
            gb = tailpsum.tile([128, 2], F32, tag="tps", name=f"gb_{p}")
            nc.tensor.matmul(gb[:, 0:1], lhsT=ind2, rhs=ginv,
                             start=True, stop=True)
            fs = rspool.tile([128, 1], F32, tag="fs", name=f"fs_{p}")
            nc.vector.tensor_mul(fs, rv, gb[:, 0:1])
            osb = tailpool.tile([128, D], F32, tag="osb", name=f"osb_{p}")
            nc.vector.tensor_scalar_mul(osb, vlad, fs)
            nc.sync.dma_start(out=outr[sa], in_=osb[0:64, :])
            nc.sync.dma_start(out=outr[sb], in_=osb[64:128, :])

        for p in range(2):
            agg = aggpsum.tile([128, D], F32, tag="agg", name=f"agg_{p}")
            cs = cspsum.tile([128, 8], F32, tag="cs", name=f"cs_{p}")
            for s in (2 * p, 2 * p + 1):
                for q in range(NQ):
                    quarter(s, q, agg, cs)
            tail_pair(p, agg, cs)

    return nc


_NC_CACHE = None


def kernel(**inputs: np.ndarray) -> np.ndarray:
    global _NC_CACHE
    _apply_patch()
    from concourse.bass_utils import run_bass_kernel_spmd

    x = np.ascontiguousarray(np.asarray(inputs["x"], dtype=np.float32))
    W = np.ascontiguousarray(np.asarray(inputs["W"], dtype=np.float32))
    cent = np.ascontiguousarray(
        np.asarray(inputs["centroids"], dtype=np.float32))

    ind2 = np.zeros((2, 128), dtype=np.float32)
    ind2[0, 0:64] = 1.0
    ind2[1, 64:128] = 1.0
    indK = np.zeros((128, 2), dtype=np.float32)
    indK[0:64, 0] = 1.0
    indK[64:128, 1] = 1.0

    if _NC_CACHE is None:
        _NC_CACHE = build_nc()
    nc = _NC_CACHE

    in_maps = [
        {
            "x": np.ascontiguousarray(
                x[B_PER_CORE * c:B_PER_CORE * (c + 1)]),
            "W": W,
            "centroids": cent,
            "ind2": ind2,
            "indK": indK,
        }
        for c in range(N_CORES)
    ]
    res = run_bass_kernel_spmd(nc, in_maps, core_ids=list(range(N_CORES)))
    return np.concatenate([r["out"] for r in res.results], axis=0)



# revision 16
# speedup vs baseline: 1.0340x; 1.0340x over previous
"""NetVLAD on 8 Trainium2 NeuronCores — self-contained kernel.

Problem: x [32, 2048, 1024] f32, W [64, 1024] f32, centroids [64, 1024] f32
  -> out [32, 65536] f32  (NetVLAD pooling: per-frame L2 norm, soft-assign
  softmax over 64 clusters, residual aggregation, intra + global L2 norm).

Sharding: data-parallel over batch — 4 samples per core, W/centroids
replicated; no cross-core communication.

Final design (~204us vs 314us baseline):
  - x ingested via SWDGE casting DMA f32->fp8e4 per quarter (read-bound at
    the HBM/SDMA ceiling; zero engine-side cast work).
  - ONE xbar DMA-transpose per quarter of the uint16(fp8-pair)-viewed data
    (half the bytes of bf16), issue split across both HWDGE queues
    (sync/scalar).
  - logits via fp8 DoubleRow matmuls (4 per m-tile, Ki=128 x Ko=2 U-blocks,
    byte offset selects the fp8 half of each transposed uint16); W goes
    through the identical cast+transpose transform so contraction pairs
    line up.
  - a (fp8) scaled x256 and invr (fp8) scaled /4 to stay in e4m3's normal
    range; the factors cancel in the tail normalizations (cs rescaled x4).
  - ssq split ACT Square+accum (2 tiles) / DVE tensor_mul+reduce (2 tiles);
    ACT restricted to {Ln, Exp, Square, Copy} (one table set).
  - BIR post-pass _overlap_dma_json: un-pins the scheduler-serialized DMA
    queues (loads/transposes overlap on the 16 SDMA engines) and paces
    loads 4 quarters ahead of transposes.
"""

import json

import numpy as np

import concourse.bass as bass
import concourse.mybir as mybir
import concourse.tile as tile

F32 = mybir.dt.float32
BF16 = mybir.dt.bfloat16
FP8 = mybir.dt.float8e4
AF = mybir.ActivationFunctionType
OP = mybir.AluOpType

B = 32
N_CORES = 8
B_PER_CORE = B // N_CORES
M = 2048
D = 1024
K = 64
NQ = 4           # quarters per sample
TQ = 4           # m-tiles per quarter

_PATCHED = False


def _prune_teardown_waits_json(bir: dict) -> dict:
    """Drop DMASW-lane waits from teardown-block Drain instructions.

    The kernel-end Drain on SP waits on every DMA lane; after wait-splitting
    each wait becomes a serial ~115ns EventSemaphore. The DMASW lanes track
    the gpsimd casting loads, every one of which was already consumed by an
    xbar transpose (whose issue waited on that lane value), so by teardown
    time those lanes are provably at their final values — the waits are pure
    dispatch overhead on the critical tail.
    """
    for f in bir.get("functions", []):
        blocks = f.get("blocks", [])
        for blk in blocks[1:]:
            for inst in blk.get("instructions", []):
                if inst.get("opcode") != "Drain":
                    continue
                si = inst.get("sync_info")
                if not si:
                    continue
                si["on_wait"] = [
                    w for w in si.get("on_wait", [])
                    if not w["ant_name"].startswith("DMASW")
                ]
    return bir


def _split_waits_json(bir: dict, max_waits: int = 1) -> dict:
    """Split multi-wait sync infos into standalone EventSemaphore waits.

    The walrus build in this image supports a single sync-wait command per
    instruction, while Tile's sem assignment emits several (e.g. the
    kernel-tail Drain waits on every DMAHW lane). Hoisting the extra waits
    into preceding single-wait EventSemaphore instructions on the same
    engine is semantics-preserving for monotonic semaphores.
    """
    ctr = 0
    for f in bir.get("functions", []):
        for blk in f.get("blocks", []):
            insts = blk.get("instructions", [])
            new = []
            for inst in insts:
                si = inst.get("sync_info")
                waits = si.get("on_wait", []) if si else []
                if len(waits) > max_waits:
                    head, keep = waits[:-max_waits], waits[-max_waits:]
                    for w in head:
                        ctr += 1
                        new.append({
                            "debug": inst.get("debug", 0),
                            "engine": inst["engine"],
                            "ins": [],
                            "name": f"{inst['name']}-wsplit{ctr}",
                            "opcode": "EventSemaphore",
                            "outs": [],
                            "sync_info": {"on_update": [], "on_wait": [w]},
                        })
                    si["on_wait"] = keep
                new.append(inst)
            blk["instructions"] = new
    return bir


def _overlap_dma_json(bir: dict) -> dict:
    """Un-serialize the x-load and xbar-transpose DMA queues.

    The Tile scheduler's cost model holds a single modeled DMA_ENGINES
    resource exclusively for every DMA, so its schedule lays all DMAs out in
    one serial chain and the semaphore pass pins that order with cross-queue
    waits (x-loads wait on recent transposes and vice versa). On hardware the
    16 SDMA engines serve both queues concurrently, so those pins only create
    idle time (measured as strict alternation of the two queues, ~30% SDMA
    idle).

    Safe rewrite, relying on xb pool bufs == number of quarters (each xb
    buffer is written exactly once, so an x-load has NO data hazard at all):
      - drop every DMAHW-lane wait from x-load DMACopy instructions (they can
        only be scheduler order-pins);
      - on each x-transpose, replace the DMASW-lane waits with the one true
        dependency: the completion value of its own quarter's x-load
        (tracked by walking lane update counts in program order). PE waits
        (xT buffer reuse) and DMAHW waits (transpose queue ordering) are
        kept.
    """
    PACE_DEPTH = 4
    for f in bir.get("functions", []):
        for blk in f.get("blocks", []):
            lane_val: dict[str, int] = {}
            load_done: dict[str, tuple[dict, int]] = {}
            tp_updates: list[tuple[dict, int]] = []
            n_loads = 0
            for inst in blk.get("instructions", []):
                si = inst.get("sync_info")
                if si is None:
                    continue
                outs = inst.get("outs") or []
                ins = inst.get("ins") or []
                outm = outs[0].get("memref", "") if outs else ""
                inm = ins[0].get("memref", "") if ins else ""
                is_xload = (
                    inst.get("opcode") == "DMACopy"
                    and inst.get("engine") == "Pool"
                    and outm.startswith("xb_")
                )
                is_xtp = (
                    inst.get("opcode") == "DmaTransposeAnt"
                    and inm.startswith("xb_")
                )
                if is_xload:
                    waits = [
                        w for w in si.get("on_wait", [])
                        if not w["ant_name"].startswith("DMAHW")
                    ]
                    # Pace the load stream: the 4KB-read load packets starve
                    # the 256B xbar packets in the SDMA round-robin, so gate
                    # load n on transpose n-PACE_DEPTH to keep the two
                    # streams in lockstep (idle SDMA slots go to transposes).
                    idx = n_loads - PACE_DEPTH
                    if 0 <= idx < len(tp_updates):
                        upd, val = tp_updates[idx]
                        waits.append({
                            "ant_name": upd["ant_name"],
                            "id": upd["id"],
                            "sync_type": "semaphore",
                            "wait_mode": "sem-ge-imm",
                            "wait_value": val,
                        })
                    si["on_wait"] = waits
                    n_loads += 1
                if is_xtp:
                    key = "_".join(inm.split("_")[:3])
                    keep = [
                        w for w in si.get("on_wait", [])
                        if not w["ant_name"].startswith("DMASW")
                    ]
                    real = load_done.get(key)
                    if real is not None:
                        upd, val = real
                        keep.append({
                            "ant_name": upd["ant_name"],
                            "id": upd["id"],
                            "sync_type": "semaphore",
                            "wait_mode": "sem-ge-imm",
                            "wait_value": val,
                        })
                    si["on_wait"] = keep
                for u in si.get("on_update", []):
                    nm = u.get("ant_name")
                    if not nm:
                        continue
                    lane_val[nm] = lane_val.get(nm, 0) + u.get(
                        "update_value", 0)
                    if is_xload:
                        key = "_".join(outm.split("_")[:3])
                        load_done[key] = (u, lane_val[nm])
                    if is_xtp:
                        tp_updates.append((u, lane_val[nm]))
    return bir


def _apply_patch():
    global _PATCHED
    if _PATCHED:
        return
    import concourse.bass_utils as bu
    import concourse.bass2jax as b2j
    orig = bu.compile_bir_kernel

    def patched(bir_json, tmpdir, neff_name="file.neff"):
        d = json.loads(bir_json)
        d = _overlap_dma_json(d)
        d = _prune_teardown_waits_json(d)
        d = _split_waits_json(d, 1)
        return orig(json.dumps(d).encode(), tmpdir, neff_name)

    bu.compile_bir_kernel = patched
    b2j.compile_bir_kernel = patched
    _PATCHED = True


def build_nc():
    nc = bass.Bass()
    x = nc.dram_tensor("x", [B_PER_CORE, M, D], F32, kind="ExternalInput")
    W = nc.dram_tensor("W", [K, D], F32, kind="ExternalInput")
    C = nc.dram_tensor("centroids", [K, D], F32, kind="ExternalInput")
    out = nc.dram_tensor("out", [B_PER_CORE, K * D], F32, kind="ExternalOutput")
    ind2_d = nc.dram_tensor("ind2", [2, 128], F32, kind="ExternalInput")
    indK_d = nc.dram_tensor("indK", [128, 2], F32, kind="ExternalInput")

    # partition p holds 4 contiguous rows (m = q*512 + 4p + j): one 16KB
    # contiguous read per partition per quarter-load instead of 4x4KB.
    # NetVLAD aggregates over m, so this row relabeling is output-invariant.
    xr = x[:, :, :].rearrange("s (q t p) d -> s q p t d", q=NQ, t=TQ, p=128)
    outr = out[:, :].rearrange("s (k d) -> s k d", d=D)

    from contextlib import ExitStack
    with tile.TileContext(nc) as tc, ExitStack() as es:
        singles = es.enter_context(tc.tile_pool(name="singles", bufs=1))
        # xb bufs=16: every quarter gets its own buffer (written exactly once)
        # so x-loads carry no write-after-read hazard at all — this makes the
        # BIR-level wait surgery in _overlap_dma_json provably safe.
        xbpool = es.enter_context(tc.tile_pool(name="xbp", bufs=16))
        xTpool = es.enter_context(tc.tile_pool(name="xTp", bufs=8))
        sqpool = es.enter_context(tc.tile_pool(name="sqp", bufs=3))
        statpool = es.enter_context(tc.tile_pool(name="statp", bufs=8))
        epool = es.enter_context(tc.tile_pool(name="ep", bufs=4))
        apool = es.enter_context(tc.tile_pool(name="apl", bufs=4))
        rspool = es.enter_context(tc.tile_pool(name="rsp", bufs=4))
        tailpool = es.enter_context(tc.tile_pool(name="tailp", bufs=1))
        zpsum = es.enter_context(tc.tile_pool(name="zps", bufs=2, space="PSUM"))
        aggpsum = es.enter_context(
            tc.tile_pool(name="aggps", bufs=2, space="PSUM"))
        # cs + tail scalars share one bank per pair (cols 0:8 cs accum,
        # 8:9 gsum, 10:11 gb) so cs can double-buffer within 8 PSUM banks:
        # pair-1's first cs matmul then never waits on pair-0's tail read.
        cspsum = es.enter_context(tc.tile_pool(name="csps", bufs=2, space="PSUM"))

        # W: fp8-cast then the same uint16-viewed xbar transpose as x, so the
        # fp8 pair layout (d = 256V + 2p + b at [p, V, :, b]) matches x's and
        # DoubleRow contraction pairs line up on both operands.
        w8 = singles.tile([K, D], FP8)
        nc.gpsimd.dma_start(out=w8, in_=W[:, :])
        WT16 = singles.tile([128, 4, K], BF16)  # [p, V, k] uint16 pairs
        nc.sync.dma_start(out=WT16, in_=w8[:, :].bitcast(BF16), transpose=True)
        cpair = singles.tile([128, D], F32)
        nc.gpsimd.dma_start(out=cpair[0:64, :], in_=C[:, :])
        nc.gpsimd.dma_start(out=cpair[64:128, :], in_=C[:, :])
        ind2 = singles.tile([2, 128], F32)
        nc.sync.dma_start(out=ind2, in_=ind2_d[:, :])
        indK = singles.tile([128, 2], F32)
        nc.sync.dma_start(out=indK, in_=indK_d[:, :])


        def quarter(s, q, agg, cs):
            base = 64 * (s % 2)
            first = q == 0
            last = q == NQ - 1

            # 1) casting quarter load (SWDGE): f32 DRAM -> fp8e4 SBUF
            #    (tile names keep the xb_ prefix: _overlap_dma_json keys on it)
            xb = xbpool.tile([128, TQ, D], FP8, tag="xb", name=f"xb_{s}_{q}")
            nc.gpsimd.dma_start(out=xb, in_=xr[s, q])
            xb_flat = xb[:, :, :].rearrange("p t d -> p (t d)")

            # 2) per-quarter xbar transpose of the uint16(=fp8 pair) view.
            #    xT16[p, tU, m] = pair (x[m, d], x[m, d+1]) with
            #    d = t*1024 + 256*U + 2p, where tU = 4t + U.
            xT16 = xTpool.tile([128, TQ, 4, 128], BF16, tag="xT",
                               name=f"xT_{s}_{q}")
            xT3 = xT16[:, :, :, :].rearrange("p t u m -> p (t u) m")
            # all transposes on the sync queue: a blocking DMA_TRANSPOSE on
            # the scalar queue stalls ACT compute (zero overlap in traces)
            nc.sync.dma_start(out=xT3, in_=xb_flat.bitcast(BF16),
                              transpose=True)

            # 3) ssq: alternate the ACT/DVE split (3/1 vs 2/2) so both
            #    engines average ~4.6us/quarter (ACT square+accum is ~2x
            #    cheaper per tile than DVE's mul+reduce pair)
            nact = 3 if (NQ * s + q) % 2 == 0 else 2
            ssq = statpool.tile([128, TQ], F32, tag="ssq", name=f"ssq_{s}_{q}")
            for i in range(nact):
                sqa = sqpool.tile([128, D], BF16, tag="sqa",
                                  name=f"sqa_{s}_{q}_{i}")
                nc.scalar.activation(
                    out=sqa, in_=xb[:, i, :], func=AF.Square,
                    accum_out=ssq[:, i:i + 1],
                )
            ndve = TQ - nact
            sqd = sqpool.tile([128, ndve, D], BF16, tag="sqd",
                              name=f"sqd_{s}_{q}")
            sqd_flat = sqd[:, :, :].rearrange("p t d -> p (t d)")
            xbd = xb[:, nact:TQ, :].rearrange("p t d -> p (t d)")
            nc.vector.tensor_mul(sqd_flat, xbd, xbd)
            nc.vector.reduce_sum(
                out=ssq[:, nact:TQ], in_=sqd, axis=mybir.AxisListType.X)

            # 4) stats: lnt/r on ACT (Ln/Exp table set); invr8 = ||x||/4 =
            #    ssq*r/4 on DVE (frees two ACT exps vs the lnt-exp route)
            lnt = statpool.tile([128, TQ], F32, tag="lnt", name=f"lnt_{s}_{q}")
            nc.scalar.activation(out=lnt, in_=ssq, func=AF.Ln)
            r = statpool.tile([128, TQ], F32, tag="r", name=f"r_{s}_{q}")
            nc.scalar.activation(out=r, in_=lnt, func=AF.Exp, scale=-0.5)
            invr8 = statpool.tile([128, TQ], FP8, tag="invr",
                                  name=f"invr_{s}_{q}")
            nc.vector.scalar_tensor_tensor(
                out=invr8, in0=ssq, scalar=0.25, in1=r,
                op0=OP.mult, op1=OP.mult,
            )

            # 5) logits z[m, k] via fp8 DoubleRow matmuls: 4 per m-tile, each
            #    contracting 256 d (Ki=128 partitions x Ko=2 U-blocks), with
            #    the byte offset b selecting the fp8 half of each uint16.
            zq = zpsum.tile([128, TQ, K], F32, tag="zq", name=f"zq_{s}_{q}")
            for i in range(TQ):
                mms = [(U0, b) for U0 in (0, 2) for b in (0, 1)]
                for j, (U0, b) in enumerate(mms):
                    lhsT = xT16[:, i, U0:U0 + 2, :].bitcast(FP8).rearrange(
                        "p u (m b) -> p u m b", b=2)[:, :, :, b]
                    rhs = WT16[:, U0:U0 + 2, :].bitcast(FP8).rearrange(
                        "p v (k b) -> p v k b", b=2)[:, :, :, b]
                    nc.tensor.matmul(
                        zq[:, i, :], lhsT=lhsT, rhs=rhs,
                        start=(j == 0), stop=(j == 3),
                        perf_mode=mybir.MatmulPerfMode.DoubleRow,
                    )

            # 6) e = exp(z * r) per tile (ACT), then one batched sden reduce
            e = epool.tile([128, TQ, K], BF16, tag="e", name=f"e_{s}_{q}")
            for i in range(TQ):
                nc.scalar.activation(
                    out=e[:, i, :], in_=zq[:, i, :], func=AF.Exp,
                    scale=r[:, i:i + 1],
                )
            sden = statpool.tile([128, TQ], F32, tag="sden",
                                 name=f"sden_{s}_{q}")
            nc.vector.reduce_sum(
                out=sden, in_=e, axis=mybir.AxisListType.X)

            # 7) a = e * (256 r / sden): one reciprocal, one fused scale
            #    s = (rsd*256)*r, one broadcast multiply over all 4 tiles
            rsd = statpool.tile([128, TQ], F32, tag="rsd", name=f"rsd_{s}_{q}")
            nc.vector.reciprocal(out=rsd, in_=sden)
            sfac = statpool.tile([128, TQ], BF16, tag="sfac",
                                 name=f"sfac_{s}_{q}")
            nc.vector.scalar_tensor_tensor(
                out=sfac, in0=rsd, scalar=256.0, in1=r,
                op0=OP.mult, op1=OP.mult,
            )
            a = apool.tile([128, TQ, K], FP8, tag="a", name=f"a_{s}_{q}")
            nc.vector.tensor_mul(
                a[:, :, :], e[:, :, :],
                sfac[:, :].unsqueeze(-1).broadcast_to((128, TQ, K)),
            )

            # 8) agg += a^T x (x256); cs += a^T (||x||/4)  (x64)
            for i in range(TQ):
                st_ = first and i == 0
                sp_ = last and i == TQ - 1
                nc.tensor.matmul(
                    agg[base:base + 64, 0:512], lhsT=a[:, i, :],
                    rhs=xb[:, i, 0:512], start=st_, stop=sp_,
                )
                nc.tensor.matmul(
                    agg[base:base + 64, 512:1024], lhsT=a[:, i, :],
                    rhs=xb[:, i, 512:1024], start=st_, stop=sp_,
                )
                nc.tensor.matmul(
                    cs[base:base + 64, 0:1], lhsT=a[:, i, :],
                    rhs=invr8[:, i:i + 1], start=st_, stop=sp_,
                )

        def tail_pair(p, agg, cs):
            sa, sb = 2 * p, 2 * p + 1
            # agg carries x256, cs carries x64 — rescale cs by 4 here so vlad
            # = 256*(agg_true - colsum*c); the factor cancels in both norms.
            cssb = rspool.tile([128, 1], F32, tag="cssb", name=f"cssb_{p}")
            nc.vector.tensor_scalar_mul(cssb, cs[:, 0:1], 4.0)
            tmp = tailpool.tile([128, D], F32, tag="tmp", name=f"tmp_{p}")
            nc.vector.tensor_scalar_mul(tmp, cpair, cssb)
            vlad = tailpool.tile([128, D], F32, tag="vlad", name=f"vlad_{p}")
            nc.vector.tensor_sub(vlad, agg[:, :], tmp)
            sq2 = sqpool.tile([128, D], BF16, tag="sqa", name=f"sqt_{p}")
            vssq = rspool.tile([128, 1], F32, tag="vssq", name=f"vssq_{p}")
            nc.scalar.activation(out=sq2, in_=vlad, func=AF.Square,
                                 accum_out=vssq)
            lnv = rspool.tile([128, 1], F32, tag="lnv", name=f"lnv_{p}")
            nc.scalar.activation(out=lnv, in_=vssq, func=AF.Ln)
            rv = rspool.tile([128, 1], F32, tag="rv", name=f"rv_{p}")
            nc.scalar.activation(out=rv, in_=lnv, func=AF.Exp, scale=-0.5)
            ssqn = rspool.tile([128, 1], F32, tag="ssqn", name=f"ssqn_{p}")
            nc.vector.tensor_scalar(
                out=ssqn, in0=vssq, scalar1=rv, scalar2=rv,
                op0=OP.mult, op1=OP.mult,
            )
            nc.tensor.matmul(cs[0:2, 8:9], lhsT=indK, rhs=ssqn,
                             start=True, stop=True)
            lng = rspool.tile([2, 1], F32, tag="lng", name=f"lng_{p}")
            nc.scalar.activation(out=lng, in_=cs[0:2, 8:9], func=AF.Ln)
            ginv = rspool.tile([2, 1], F32, tag="ginv", name=f"ginv_{p}")
            nc.scalar.activation(out=ginv, in_=lng, func=AF.Exp, scale=-0.5)
            nc.tensor.matmul(cs[:, 10:11], lhsT=ind2, rhs=ginv,
                             start=True, stop=True)
            fs = rspool.tile([128, 1], F32, tag="fs", name=f"fs_{p}")
            nc.vector.tensor_mul(fs, rv, cs[:, 10:11])
            osb = tailpool.tile([128, D], F32, tag="osb", name=f"osb_{p}")
            nc.vector.tensor_scalar_mul(osb, vlad, fs)
            nc.sync.dma_start(out=outr[sa], in_=osb[0:64, :])
            nc.sync.dma_start(out=outr[sb], in_=osb[64:128, :])

        # Emission order defers pair-0's tail until after sample 2: the tail's
        # ACT ops (Square/Ln/Exp chain) land behind pair-1 quarter work in the
        # strict-FIFO ACT queue, so their late-resolving DVE deps no longer
        # stall pair-1's squares/exps (measured 25us priority inversion).
        pairs = []
        for p in range(2):
            agg = aggpsum.tile([128, D], F32, tag="agg", name=f"agg_{p}")
            cs = cspsum.tile([128, 16], F32, tag="cs", name=f"cs_{p}")
            pairs.append((agg, cs))
        for q in range(NQ):
            quarter(0, q, *pairs[0])
        for q in range(NQ):
            quarter(1, q, *pairs[0])
        for q in range(NQ):
            quarter(2, q, *pairs[1])
        tail_pair(0, *pairs[0])
        for q in range(NQ):
            quarter(3, q, *pairs[1])
        tail_pair(1, *pairs[1])

    return nc


_NC_CACHE = None


def kernel(**inputs: np.ndarray) -> np.ndarray:
    global _NC_CACHE
    _apply_patch()
    from concourse.bass_utils import run_bass_kernel_spmd

    x = np.ascontiguousarray(np.asarray(inputs["x"], dtype=np.float32))
    W = np.ascontiguousarray(np.asarray(inputs["W"], dtype=np.float32))
    cent = np.ascontiguousarray(
        np.asarray(inputs["centroids"], dtype=np.float32))

    ind2 = np.zeros((2, 128), dtype=np.float32)
    ind2[0, 0:64] = 1.0
    ind2[1, 64:128] = 1.0
    indK = np.zeros((128, 2), dtype=np.float32)
    indK[0:64, 0] = 1.0
    indK[64:128, 1] = 1.0

    if _NC_CACHE is None:
        _NC_CACHE = build_nc()
    nc = _NC_CACHE

    in_maps = [
        {
            "x": np.ascontiguousarray(
                x[B_PER_CORE * c:B_PER_CORE * (c + 1)]),
            "W": W,
            "centroids": cent,
            "ind2": ind2,
            "indK": indK,
        }
        for c in range(N_CORES)
    ]
    res = run_bass_kernel_spmd(nc, in_maps, core_ids=list(range(N_CORES)))
    return np.concatenate([r["out"] for r in res.results], axis=0)



# revision 24
# speedup vs baseline: 1.0972x; 1.0611x over previous
"""NetVLAD on 8 Trainium2 NeuronCores — self-contained kernel.

Problem: x [32, 2048, 1024] f32, W [64, 1024] f32, centroids [64, 1024] f32
  -> out [32, 65536] f32  (NetVLAD pooling: per-frame L2 norm, soft-assign
  softmax over 64 clusters, residual aggregation, intra + global L2 norm).

Sharding: data-parallel over batch — 4 samples per core, W/centroids
replicated; no cross-core communication.

Final design (~204us vs 314us baseline):
  - x ingested via SWDGE casting DMA f32->fp8e4 per quarter (read-bound at
    the HBM/SDMA ceiling; zero engine-side cast work).
  - ONE xbar DMA-transpose per quarter of the uint16(fp8-pair)-viewed data
    (half the bytes of bf16), issue split across both HWDGE queues
    (sync/scalar).
  - logits via fp8 DoubleRow matmuls (4 per m-tile, Ki=128 x Ko=2 U-blocks,
    byte offset selects the fp8 half of each transposed uint16); W goes
    through the identical cast+transpose transform so contraction pairs
    line up.
  - a (fp8) scaled x256 and invr (fp8) scaled /4 to stay in e4m3's normal
    range; the factors cancel in the tail normalizations (cs rescaled x4).
  - ssq split ACT Square+accum (2 tiles) / DVE tensor_mul+reduce (2 tiles);
    ACT restricted to {Ln, Exp, Square, Copy} (one table set).
  - BIR post-pass _overlap_dma_json: un-pins the scheduler-serialized DMA
    queues (loads/transposes overlap on the 16 SDMA engines) and paces
    loads 4 quarters ahead of transposes.
"""

import json

import numpy as np

import concourse.bass as bass
import concourse.mybir as mybir
import concourse.tile as tile

F32 = mybir.dt.float32
BF16 = mybir.dt.bfloat16
FP8 = mybir.dt.float8e4
AF = mybir.ActivationFunctionType
OP = mybir.AluOpType

B = 32
N_CORES = 8
B_PER_CORE = B // N_CORES
M = 2048
D = 1024
K = 64
NQ = 4           # quarters per sample
TQ = 4           # m-tiles per quarter

_PATCHED = False


def _prune_teardown_waits_json(bir: dict) -> dict:
    """Drop DMASW-lane waits from teardown-block Drain instructions.

    The kernel-end Drain on SP waits on every DMA lane; after wait-splitting
    each wait becomes a serial ~115ns EventSemaphore. The DMASW lanes track
    the gpsimd casting loads, every one of which was already consumed by an
    xbar transpose (whose issue waited on that lane value), so by teardown
    time those lanes are provably at their final values — the waits are pure
    dispatch overhead on the critical tail.
    """
    for f in bir.get("functions", []):
        blocks = f.get("blocks", [])
        for blk in blocks[1:]:
            for inst in blk.get("instructions", []):
                if inst.get("opcode") != "Drain":
                    continue
                si = inst.get("sync_info")
                if not si:
                    continue
                si["on_wait"] = [
                    w for w in si.get("on_wait", [])
                    if not w["ant_name"].startswith("DMASW")
                ]
    return bir


def _split_waits_json(bir: dict, max_waits: int = 1) -> dict:
    """Split multi-wait sync infos into standalone EventSemaphore waits.

    The walrus build in this image supports a single sync-wait command per
    instruction, while Tile's sem assignment emits several (e.g. the
    kernel-tail Drain waits on every DMAHW lane). Hoisting the extra waits
    into preceding single-wait EventSemaphore instructions on the same
    engine is semantics-preserving for monotonic semaphores.
    """
    ctr = 0
    for f in bir.get("functions", []):
        for blk in f.get("blocks", []):
            insts = blk.get("instructions", [])
            new = []
            for inst in insts:
                si = inst.get("sync_info")
                waits = si.get("on_wait", []) if si else []
                if len(waits) > max_waits:
                    head, keep = waits[:-max_waits], waits[-max_waits:]
                    for w in head:
                        ctr += 1
                        new.append({
                            "debug": inst.get("debug", 0),
                            "engine": inst["engine"],
                            "ins": [],
                            "name": f"{inst['name']}-wsplit{ctr}",
                            "opcode": "EventSemaphore",
                            "outs": [],
                            "sync_info": {"on_update": [], "on_wait": [w]},
                        })
                    si["on_wait"] = keep
                new.append(inst)
            blk["instructions"] = new
    return bir


def _overlap_dma_json(bir: dict) -> dict:
    """Un-serialize the x-load and xbar-transpose DMA queues.

    The Tile scheduler's cost model holds a single modeled DMA_ENGINES
    resource exclusively for every DMA, so its schedule lays all DMAs out in
    one serial chain and the semaphore pass pins that order with cross-queue
    waits (x-loads wait on recent transposes and vice versa). On hardware the
    16 SDMA engines serve both queues concurrently, so those pins only create
    idle time (measured as strict alternation of the two queues, ~30% SDMA
    idle).

    Safe rewrite, relying on xb pool bufs == number of quarters (each xb
    buffer is written exactly once, so an x-load has NO data hazard at all):
      - drop every DMAHW-lane wait from x-load DMACopy instructions (they can
        only be scheduler order-pins);
      - on each x-transpose, replace the DMASW-lane waits with the one true
        dependency: the completion value of its own quarter's x-load
        (tracked by walking lane update counts in program order). PE waits
        (xT buffer reuse) and DMAHW waits (transpose queue ordering) are
        kept.
    """
    PACE_DEPTH = 4
    for f in bir.get("functions", []):
        for blk in f.get("blocks", []):
            lane_val: dict[str, int] = {}
            load_done: dict[str, tuple[dict, int]] = {}
            tp_updates: list[tuple[dict, int]] = []
            n_loads = 0
            for inst in blk.get("instructions", []):
                si = inst.get("sync_info")
                if si is None:
                    continue
                outs = inst.get("outs") or []
                ins = inst.get("ins") or []
                outm = outs[0].get("memref", "") if outs else ""
                inm = ins[0].get("memref", "") if ins else ""
                is_xload = (
                    inst.get("opcode") == "DMACopy"
                    and inst.get("engine") == "Pool"
                    and outm.startswith("xb_")
                )
                is_wload = (
                    inst.get("opcode") == "DMACopy"
                    and inst.get("engine") == "Pool"
                    and outm.startswith("w8")
                )
                is_xtp = (
                    inst.get("opcode") == "DmaTransposeAnt"
                    and inm.startswith("xb_")
                )
                is_wtp = (
                    inst.get("opcode") == "DmaTransposeAnt"
                    and inm.startswith("w8")
                )
                if is_xload:
                    waits = [
                        w for w in si.get("on_wait", [])
                        if not w["ant_name"].startswith("DMAHW")
                    ]
                    # Pace the load stream: the 4KB-read load packets starve
                    # the 256B xbar packets in the SDMA round-robin, so gate
                    # load n on transpose n-PACE_DEPTH to keep the two
                    # streams in lockstep (idle SDMA slots go to transposes).
                    idx = n_loads - PACE_DEPTH
                    if 0 <= idx < len(tp_updates):
                        upd, val = tp_updates[idx]
                        waits.append({
                            "ant_name": upd["ant_name"],
                            "id": upd["id"],
                            "sync_type": "semaphore",
                            "wait_mode": "sem-ge-imm",
                            "wait_value": val,
                        })
                    si["on_wait"] = waits
                    n_loads += 1
                if is_xtp or is_wtp:
                    # true dep: the completion value of this tile's own load
                    key = "w8" if is_wtp else "_".join(inm.split("_")[:3])
                    keep = [
                        w for w in si.get("on_wait", [])
                        if not w["ant_name"].startswith("DMASW")
                    ]
                    real = load_done.get(key)
                    if real is not None:
                        upd, val = real
                        keep.append({
                            "ant_name": upd["ant_name"],
                            "id": upd["id"],
                            "sync_type": "semaphore",
                            "wait_mode": "sem-ge-imm",
                            "wait_value": val,
                        })
                    si["on_wait"] = keep
                for u in si.get("on_update", []):
                    nm = u.get("ant_name")
                    if not nm:
                        continue
                    lane_val[nm] = lane_val.get(nm, 0) + u.get(
                        "update_value", 0)
                    if is_xload:
                        key = "_".join(outm.split("_")[:3])
                        load_done[key] = (u, lane_val[nm])
                    if is_wload:
                        load_done["w8"] = (u, lane_val[nm])
                    if is_xtp:
                        tp_updates.append((u, lane_val[nm]))
    return bir


def _apply_patch():
    global _PATCHED
    if _PATCHED:
        return
    import concourse.bass_utils as bu
    import concourse.bass2jax as b2j
    orig = bu.compile_bir_kernel

    def patched(bir_json, tmpdir, neff_name="file.neff"):
        d = json.loads(bir_json)
        d = _overlap_dma_json(d)
        d = _prune_teardown_waits_json(d)
        d = _split_waits_json(d, 1)
        return orig(json.dumps(d).encode(), tmpdir, neff_name)

    bu.compile_bir_kernel = patched
    b2j.compile_bir_kernel = patched
    _PATCHED = True


def build_nc():
    nc = bass.Bass()
    x = nc.dram_tensor("x", [B_PER_CORE, M, D], F32, kind="ExternalInput")
    W = nc.dram_tensor("W", [K, D], F32, kind="ExternalInput")
    C = nc.dram_tensor("centroids", [K, D], F32, kind="ExternalInput")
    out = nc.dram_tensor("out", [B_PER_CORE, K * D], F32, kind="ExternalOutput")

    # partition p holds 4 contiguous rows (m = q*512 + 4p + j): one 16KB
    # contiguous read per partition per quarter-load instead of 4x4KB.
    # NetVLAD aggregates over m, so this row relabeling is output-invariant.
    xr = x[:, :, :].rearrange("s (q t p) d -> s q p t d", q=NQ, t=TQ, p=128)
    outr = out[:, :].rearrange("s (k d) -> s k d", d=D)

    from contextlib import ExitStack
    with tile.TileContext(nc) as tc, ExitStack() as es:
        singles = es.enter_context(tc.tile_pool(name="singles", bufs=1))
        # xb bufs=16: every quarter gets its own buffer (written exactly once)
        # so x-loads carry no write-after-read hazard at all — this makes the
        # BIR-level wait surgery in _overlap_dma_json provably safe.
        xbpool = es.enter_context(tc.tile_pool(name="xbp", bufs=16))
        xTpool = es.enter_context(tc.tile_pool(name="xTp", bufs=8))
        sqpool = es.enter_context(tc.tile_pool(name="sqp", bufs=3))
        statpool = es.enter_context(tc.tile_pool(name="statp", bufs=8))
        epool = es.enter_context(tc.tile_pool(name="ep", bufs=4))
        apool = es.enter_context(tc.tile_pool(name="apl", bufs=4))
        rspool = es.enter_context(tc.tile_pool(name="rsp", bufs=4))
        tailpool = es.enter_context(tc.tile_pool(name="tailp", bufs=1))
        zpsum = es.enter_context(tc.tile_pool(name="zps", bufs=2, space="PSUM"))
        aggpsum = es.enter_context(
            tc.tile_pool(name="aggps", bufs=2, space="PSUM"))
        # cs + tail scalars share one bank per pair (cols 0:8 cs accum,
        # 8:9 gsum, 10:11 gb) so cs can double-buffer within 8 PSUM banks:
        # pair-1's first cs matmul then never waits on pair-0's tail read.
        cspsum = es.enter_context(tc.tile_pool(name="csps", bufs=2, space="PSUM"))

        # W: fp8-cast then the same uint16-viewed xbar transpose as x, so the
        # fp8 pair layout (d = 256V + 2p + b at [p, V, :, b]) matches x's and
        # DoubleRow contraction pairs line up on both operands.
        w8 = singles.tile([K, D], FP8)
        nc.gpsimd.dma_start(out=w8, in_=W[:, :])
        WT16 = singles.tile([128, 4, K], BF16)  # [p, V, k] uint16 pairs
        nc.sync.dma_start(out=WT16, in_=w8[:, :].bitcast(BF16), transpose=True)
        cpair = singles.tile([128, D], F32)
        nc.gpsimd.dma_start(out=cpair[0:64, :], in_=C[:, :])
        nc.gpsimd.dma_start(out=cpair[64:128, :], in_=C[:, :])
        bias_g = singles.tile([128, 1], F32)    # ln(1/8), global-norm fold
        nc.vector.memset(bias_g, -2.0794415416798357)


        def quarter(s, q, agg, cs):
            base = 64 * (s % 2)
            first = q == 0
            last = q == NQ - 1

            # 1) casting quarter load (SWDGE): f32 DRAM -> fp8e4 SBUF
            #    (tile names keep the xb_ prefix: _overlap_dma_json keys on it)
            xb = xbpool.tile([128, TQ, D], FP8, tag="xb", name=f"xb_{s}_{q}")
            nc.gpsimd.dma_start(out=xb, in_=xr[s, q])
            xb_flat = xb[:, :, :].rearrange("p t d -> p (t d)")

            # 2) per-quarter xbar transpose of the uint16(=fp8 pair) view.
            #    xT16[p, tU, m] = pair (x[m, d], x[m, d+1]) with
            #    d = t*1024 + 256*U + 2p, where tU = 4t + U.
            xT16 = xTpool.tile([128, TQ, 4, 128], BF16, tag="xT",
                               name=f"xT_{s}_{q}")
            xT3 = xT16[:, :, :, :].rearrange("p t u m -> p (t u) m")
            # all transposes on the sync queue: a blocking DMA_TRANSPOSE on
            # the scalar queue stalls ACT compute (zero overlap in traces)
            nc.sync.dma_start(out=xT3, in_=xb_flat.bitcast(BF16),
                              transpose=True)

            # 3) ssq: alternate the ACT/DVE split (3/1 vs 2/2) so both
            #    engines average ~4.6us/quarter (ACT square+accum is ~2x
            #    cheaper per tile than DVE's mul+reduce pair)
            nact = 3 if (NQ * s + q) % 2 == 0 else 2
            ssq = statpool.tile([128, TQ], F32, tag="ssq", name=f"ssq_{s}_{q}")
            for i in range(nact):
                sqa = sqpool.tile([128, D], BF16, tag="sqa",
                                  name=f"sqa_{s}_{q}_{i}")
                nc.scalar.activation(
                    out=sqa, in_=xb[:, i, :], func=AF.Square,
                    accum_out=ssq[:, i:i + 1],
                )
            ndve = TQ - nact
            sqd = sqpool.tile([128, ndve, D], BF16, tag="sqd",
                              name=f"sqd_{s}_{q}")
            sqd_flat = sqd[:, :, :].rearrange("p t d -> p (t d)")
            xbd = xb[:, nact:TQ, :].rearrange("p t d -> p (t d)")
            nc.vector.tensor_mul(sqd_flat, xbd, xbd)
            nc.vector.reduce_sum(
                out=ssq[:, nact:TQ], in_=sqd, axis=mybir.AxisListType.X)

            # 4) stats: lnt/r on ACT (Ln/Exp table set); invr8 = ||x||/4 =
            #    ssq*r/4 on DVE (frees two ACT exps vs the lnt-exp route)
            lnt = statpool.tile([128, TQ], F32, tag="lnt", name=f"lnt_{s}_{q}")
            nc.scalar.activation(out=lnt, in_=ssq, func=AF.Ln)
            r = statpool.tile([128, TQ], F32, tag="r", name=f"r_{s}_{q}")
            nc.scalar.activation(out=r, in_=lnt, func=AF.Exp, scale=-0.5)
            invr8 = statpool.tile([128, TQ], FP8, tag="invr",
                                  name=f"invr_{s}_{q}")
            nc.vector.scalar_tensor_tensor(
                out=invr8, in0=ssq, scalar=0.25, in1=r,
                op0=OP.mult, op1=OP.mult,
            )

            # 5) logits z[m, k] via fp8 DoubleRow matmuls: 4 per m-tile, each
            #    contracting 256 d (Ki=128 partitions x Ko=2 U-blocks), with
            #    the byte offset b selecting the fp8 half of each uint16.
            zq = zpsum.tile([128, TQ, K], F32, tag="zq", name=f"zq_{s}_{q}")
            for i in range(TQ):
                mms = [(U0, b) for U0 in (0, 2) for b in (0, 1)]
                for j, (U0, b) in enumerate(mms):
                    lhsT = xT16[:, i, U0:U0 + 2, :].bitcast(FP8).rearrange(
                        "p u (m b) -> p u m b", b=2)[:, :, :, b]
                    rhs = WT16[:, U0:U0 + 2, :].bitcast(FP8).rearrange(
                        "p v (k b) -> p v k b", b=2)[:, :, :, b]
                    nc.tensor.matmul(
                        zq[:, i, :], lhsT=lhsT, rhs=rhs,
                        start=(j == 0), stop=(j == 3),
                        perf_mode=mybir.MatmulPerfMode.DoubleRow,
                    )

            # 6) e = exp(z * r) per tile (ACT), then one batched sden reduce
            e = epool.tile([128, TQ, K], BF16, tag="e", name=f"e_{s}_{q}")
            for i in range(TQ):
                nc.scalar.activation(
                    out=e[:, i, :], in_=zq[:, i, :], func=AF.Exp,
                    scale=r[:, i:i + 1],
                )
            sden = statpool.tile([128, TQ], F32, tag="sden",
                                 name=f"sden_{s}_{q}")
            nc.vector.reduce_sum(
                out=sden, in_=e, axis=mybir.AxisListType.X)

            # 7) a = e * (256 r / sden): one reciprocal, one fused scale
            #    s = (rsd*256)*r, one broadcast multiply over all 4 tiles
            rsd = statpool.tile([128, TQ], F32, tag="rsd", name=f"rsd_{s}_{q}")
            nc.vector.reciprocal(out=rsd, in_=sden)
            sfac = statpool.tile([128, TQ], BF16, tag="sfac",
                                 name=f"sfac_{s}_{q}")
            nc.vector.scalar_tensor_tensor(
                out=sfac, in0=rsd, scalar=256.0, in1=r,
                op0=OP.mult, op1=OP.mult,
            )
            a = apool.tile([128, TQ, K], FP8, tag="a", name=f"a_{s}_{q}")
            nc.vector.tensor_mul(
                a[:, :, :], e[:, :, :],
                sfac[:, :].unsqueeze(-1).broadcast_to((128, TQ, K)),
            )

            # 8) agg += a^T x (x256); cs += a^T (||x||/4)  (x64)
            for i in range(TQ):
                st_ = first and i == 0
                sp_ = last and i == TQ - 1
                nc.tensor.matmul(
                    agg[base:base + 64, 0:512], lhsT=a[:, i, :],
                    rhs=xb[:, i, 0:512], start=st_, stop=sp_,
                )
                nc.tensor.matmul(
                    agg[base:base + 64, 512:1024], lhsT=a[:, i, :],
                    rhs=xb[:, i, 512:1024], start=st_, stop=sp_,
                )
                nc.tensor.matmul(
                    cs[base:base + 64, 0:1], lhsT=a[:, i, :],
                    rhs=invr8[:, i:i + 1], start=st_, stop=sp_,
                )

        def tail_pair(p, agg, cs):
            sa, sb = 2 * p, 2 * p + 1
            # agg carries x256, cs carries x64 — rescale cs by 4 here so vlad
            # = 256*(agg_true - colsum*c); the factor cancels in both norms.
            cssb = rspool.tile([128, 1], F32, tag="cssb", name=f"cssb_{p}")
            nc.vector.tensor_scalar_mul(cssb, cs[:, 0:1], 4.0)
            tmp = tailpool.tile([128, D], F32, tag="tmp", name=f"tmp_{p}")
            nc.vector.tensor_scalar_mul(tmp, cpair, cssb)
            vlad = tailpool.tile([128, D], F32, tag="vlad", name=f"vlad_{p}")
            nc.vector.tensor_sub(vlad, agg[:, :], tmp)
            sq2 = sqpool.tile([128, D], BF16, tag="sqa", name=f"sqt_{p}")
            vssq = rspool.tile([128, 1], F32, tag="vssq", name=f"vssq_{p}")
            nc.scalar.activation(out=sq2, in_=vlad, func=AF.Square,
                                 accum_out=vssq)
            lnv = rspool.tile([128, 1], F32, tag="lnv", name=f"lnv_{p}")
            nc.scalar.activation(out=lnv, in_=vssq, func=AF.Ln)
            # After intra-normalization every cluster row has unit L2 norm,
            # so the global norm over K=64 rows is exactly sqrt(64)=8 (any
            # per-row scale cancels in vssq*rv^2=1). Fold the 1/8 into the
            # rv exponent: fs = vssq^-0.5 / 8 = exp(-0.5 lnv + ln(1/8)).
            fs = rspool.tile([128, 1], F32, tag="fs", name=f"fs_{p}")
            nc.scalar.activation(out=fs, in_=lnv, func=AF.Exp, scale=-0.5,
                                 bias=bias_g[:, 0:1])
            osb = tailpool.tile([128, D], F32, tag="osb", name=f"osb_{p}")
            nc.vector.tensor_scalar_mul(osb, vlad, fs)
            nc.sync.dma_start(out=outr[sa], in_=osb[0:64, :])
            nc.sync.dma_start(out=outr[sb], in_=osb[64:128, :])

        # Emission order defers pair-0's tail until after sample 2: the tail's
        # ACT ops (Square/Ln/Exp chain) land behind pair-1 quarter work in the
        # strict-FIFO ACT queue, so their late-resolving DVE deps no longer
        # stall pair-1's squares/exps (measured 25us priority inversion).
        pairs = []
        for p in range(2):
            agg = aggpsum.tile([128, D], F32, tag="agg", name=f"agg_{p}")
            cs = cspsum.tile([128, 16], F32, tag="cs", name=f"cs_{p}")
            pairs.append((agg, cs))
        for q in range(NQ):
            quarter(0, q, *pairs[0])
        for q in range(NQ):
            quarter(1, q, *pairs[0])
        for q in range(NQ):
            quarter(2, q, *pairs[1])
        tail_pair(0, *pairs[0])
        for q in range(NQ):
            quarter(3, q, *pairs[1])
        tail_pair(1, *pairs[1])

    return nc


_NC_CACHE = None


def kernel(**inputs: np.ndarray) -> np.ndarray:
    global _NC_CACHE
    _apply_patch()
    from concourse.bass_utils import run_bass_kernel_spmd

    x = np.ascontiguousarray(np.asarray(inputs["x"], dtype=np.float32))
    W = np.ascontiguousarray(np.asarray(inputs["W"], dtype=np.float32))
    cent = np.ascontiguousarray(
        np.asarray(inputs["centroids"], dtype=np.float32))

    if _NC_CACHE is None:
        _NC_CACHE = build_nc()
    nc = _NC_CACHE

    in_maps = [
        {
            "x": np.ascontiguousarray(
                x[B_PER_CORE * c:B_PER_CORE * (c + 1)]),
            "W": W,
            "centroids": cent,
        }
        for c in range(N_CORES)
    ]
    res = run_bass_kernel_spmd(nc, in_maps, core_ids=list(range(N_CORES)))
    return np.concatenate([r["out"] for r in res.results], axis=0)

